# revision 1
# baseline (speedup 1.0000x reference)
"""GCN classifier (3-layer GCNConv + residual + leaky_relu + global mean pool)
as a Bass/Tile kernel on 8 Trainium2 NeuronCores.

Sharding: nodes are range-partitioned across the 8 cores (6250 each, padded
to 6656); each core owns all edges whose destination lands in its range
(self-loops are materialized as explicit edges, which makes the GCN self-loop
term fall out of the same aggregation). Per layer, each core:
  - dma_gathers the 256B feature rows y[src] (y = x * deg^-1/2, the halo
    exchange tensor) from a DRAM replica filled by an AllGather,
  - segment-sums them into its own nodes with PE indicator matmuls
    (indicator[e, n] = (dst_rel[e] == n) built on DVE via broadcast compare),
  - applies dst-side deg^-1/2, the shared 64x64 weight, bias, residual and
    leaky_relu, and AllGathers the rescaled result for the next layer.
Integer in-degree counts fall out of the host-side edge partitioning
(np.bincount over dst, the same bookkeeping that builds the per-tile chunk
plan); deg^-1/2 itself (max/sqrt/reciprocal) is computed on device. The final
global-mean-pool partials (feature sums + counts per graph) are computed with
one more indicator matmul; the host sums the 8 partials and divides.
A device-side degree pass is kept behind DEG_ON_HOST=False.
"""

import numpy as np

N = 50000
D = 64
G = 64
L = 3
C = 8
NPC = N // C            # 6250 real nodes per core
TIL = 64                # indicator width / node tile
GRP = 512               # nodes per PSUM group
NPC_PAD = 6656          # 13 * 512 = 52 * 128
NT = NPC_PAD // TIL     # 104 tiles
NGRP = NPC_PAD // GRP   # 13
TPG = GRP // TIL        # 8 tiles per group
HALF = C // 2 * NPC_PAD  # 26624 — first 4 cores' rows
PAD_DST = -1000.0
LRELU_DECOMP = False  # sim-only: bass_interp lacks Lrelu; decompose via Relu
TRACE = False         # test-only: capture NTFF profile, report exec_time_ns
LAST_RESULT = None    # test-only: BassKernelResults of the last run
SKIP_GATHER = False   # perf-probe: replace dma_gather with memset
SKIP_DEG = False      # perf-probe: dinv := 1 instead of degree pass
DEG_ON_HOST = True    # feed integer degree counts from host prep; rsqrt on device
SKIP_IND = False      # perf-probe: indicators via memset instead of is_equal
SKIP_AGG = False      # perf-probe: skip aggregation matmuls
NLAYERS = L           # perf-probe: layer count override
COL_PACK = True       # pack chunk pairs into the two PE column halves
GATHER_SPLIT = 1      # sub-gathers per (group, half) batch
STOP_AFTER = ""       # perf-probe: truncate program after phase
                      # ("setup", "deg", "y0", "L0", "L1", "L2")


def _host_prep(x, edge_index, batch):
    src = np.asarray(edge_index[0], dtype=np.int64)
    dst = np.asarray(edge_index[1], dtype=np.int64)
    # self loops as explicit edges
    loops = np.arange(N, dtype=np.int64)
    src = np.concatenate([src, loops])
    dst = np.concatenate([dst, loops])

    # padded global row id in the AllGather buffer
    rows = (src // NPC) * NPC_PAD + (src % NPC)
    half = (rows >= HALF).astype(np.int64)
    lrow = rows - half * HALF  # local row within its half, < 26624

    core = dst // NPC
    dloc = dst % NPC
    tile = dloc // TIL
    drel = dloc % TIL

    order = np.lexsort((half, tile, core))
    core_s, tile_s, half_s = core[order], tile[order], half[order]
    lrow_s, drel_s = lrow[order], drel[order]

    key = (core_s * NT + tile_s) * 2 + half_s
    cnt = np.bincount(key, minlength=C * NT * 2).reshape(C, NT, 2)
    chunks = -(-cnt // 128)  # ceil div per (core, tile, half)
    plan = chunks.max(axis=0)          # [NT, 2] — shared across cores
    plan[:, 0] = np.maximum(plan[:, 0], 1)

    starts = np.zeros(C * NT * 2 + 1, np.int64)
    np.cumsum(cnt.reshape(-1), out=starts[1:])

    tot_chunks = int(plan.sum())
    tot_idx = tot_chunks * 128
    gidx = np.zeros((C, tot_idx), np.int16)
    dstrel = np.full((C, tot_chunks * 128), PAD_DST, np.float32)

    batch_chunks = np.zeros((NGRP, 2), np.int64)
    for g in range(NGRP):
        for h in range(2):
            batch_chunks[g, h] = plan[g * TPG:(g + 1) * TPG, h].sum()

    # fill per-core data in batch layout: for g, for h, for t in tiles(g)
    ci = 0
    for g in range(NGRP):
        for h in range(2):
            for tt in range(TPG):
                t = g * TPG + tt
                nch = int(plan[t, h])
                for c in range(C):
                    s = starts[(c * NT + t) * 2 + h]
                    e = starts[(c * NT + t) * 2 + h + 1]
                    n = e - s
                    gidx[c, ci * 128: ci * 128 + n] = lrow_s[s:e]
                    dstrel[c, ci * 128: ci * 128 + n] = drel_s[s:e]
                ci += nch
    assert ci == tot_chunks

    # wrap gather indices per batch block: logical i -> [i % 16, i // 16]
    gidx_w = np.zeros((C, 128, tot_idx // 16), np.int16)
    col = 0
    for g in range(NGRP):
        for h in range(2):
            nb = int(batch_chunks[g, h]) * 128
            blk = gidx[:, col * 16:col * 16 + nb].reshape(C, nb // 16, 16)
            gidx_w[:, :16, col:col + nb // 16] = np.transpose(blk, (0, 2, 1))
            col += nb // 16
    gidx_w = np.tile(gidx_w[:, :16, :], (1, 8, 1))

    dstrel_w = np.ascontiguousarray(
        dstrel.reshape(C, tot_chunks, 128).transpose(0, 2, 1))  # [C,128,TOTC]

    # integer in-degree counts per padded local node (self-loops included),
    # node-major [128, NPC_PAD//128] so deg -> rsqrt uses all 128 DVE lanes
    degs = []
    dl = np.bincount(dst, minlength=N).astype(np.float32)
    for c in range(C):
        dp = np.zeros(NPC_PAD, np.float32)
        dp[:NPC] = dl[c * NPC:(c + 1) * NPC]
        degs.append(dp.reshape(NPC_PAD // 128, 128).T.copy())  # [128, 52]

    xs, bvs = [], []
    b = np.asarray(batch, dtype=np.int64)
    for c in range(C):
        xp = np.zeros((NPC_PAD, D), np.float32)
        xp[:NPC] = np.asarray(x[c * NPC:(c + 1) * NPC], np.float32)
        xs.append(xp)
        bv = np.full(NPC_PAD, PAD_DST, np.float32)
        bv[:NPC] = b[c * NPC:(c + 1) * NPC].astype(np.float32)
        bvs.append(bv.reshape(NPC_PAD // 128, 128).T.copy())  # [128, 52]
    return xs, bvs, gidx_w, dstrel_w, batch_chunks, plan, tot_chunks, degs


_BUILD_CACHE = {}


def _build(batch_chunks, plan, tot_chunks):
    import concourse.bacc as bacc
    import concourse.tile as tile
    import concourse.mybir as mybir

    f32 = mybir.dt.float32
    TOTC = tot_chunks
    MAXCH = int(batch_chunks.max())
    AF = mybir.ActivationFunctionType
    ALU = mybir.AluOpType

    nc = bacc.Bacc("TRN2", target_bir_lowering=False, debug=False, num_devices=C)

    _ORDER = ["setup", "deg", "y0", "L0", "L1", "L2", "pool"]

    def _runs(stage):
        if not STOP_AFTER:
            return True
        return _ORDER.index(stage) <= _ORDER.index(STOP_AFTER)

    iota_c = nc.inline_tensor(
        np.tile(np.arange(TIL, dtype=np.float32)[None, :], (128, 1)), name="iota_c")
    id_c = nc.inline_tensor(np.eye(128, dtype=np.float32), name="id_c")
    ones_col_c = nc.inline_tensor(np.ones((128, 1), np.float32), name="ones_col_c")
    ones_row_c = nc.inline_tensor(np.ones((1, 512), np.float32), name="ones_row_c")

    # chunk/idx col base per (g, h) batch
    cbase = np.zeros((NGRP, 2), np.int64)
    acc = 0
    for g in range(NGRP):
        for h in range(2):
            cbase[g, h] = acc
            acc += int(batch_chunks[g, h])
    # chunk offset of tile tt within batch (g, h)
    toff = np.zeros((NGRP, 2, TPG), np.int64)
    for g in range(NGRP):
        for h in range(2):
            o = 0
            for tt in range(TPG):
                toff[g, h, tt] = o
                o += int(plan[g * TPG + tt, h])

    with tile.TileContext(nc) as tc:
        with tc.tile_pool(name="dram", bufs=1, space="DRAM") as dram, \
             tc.tile_pool(name="per", bufs=1) as per, \
             tc.tile_pool(name="wrk", bufs=3) as wrk, \
             tc.tile_pool(name="sml", bufs=2) as sml, \
             tc.tile_pool(name="ps", bufs=2, space="PSUM") as ps:

            x_own = dram.tile([NPC_PAD, D], f32, kind="ExternalInput", name="x_own", uniquify=False)
            gidx_t = dram.tile([128, TOTC * 8], mybir.dt.int16, kind="ExternalInput", name="gidx", uniquify=False)
            dstrel_t = dram.tile([128, TOTC], f32, kind="ExternalInput", name="dstrel", uniquify=False)
            batchv_t = dram.tile([128, NPC_PAD // 128], f32, kind="ExternalInput", name="batchv", uniquify=False)
            Ws_t = dram.tile([L, D, D], f32, kind="ExternalInput", name="Ws", uniquify=False)
            bs_t = dram.tile([L, D], f32, kind="ExternalInput", name="bs", uniquify=False)
            out_t = dram.tile([D + 1, G], f32, kind="ExternalOutput", name="out_partial", uniquify=False)
            deg_t = dram.tile([128, NPC_PAD // 128], f32, kind="ExternalInput", name="deg_own", uniquify=False)

            y_shard = [dram.tile([NPC_PAD, D], f32, kind="Internal", name=f"y_shard{l}")
                       for l in range(L)]
            y_full = [dram.tile([C * NPC_PAD, D], f32, kind="Internal",
                                addr_space="Shared", name=f"y_full{l}")
                      for l in range(L)]
            dinv_dram = dram.tile([NPC_PAD // 128, 128], f32, kind="Internal", name="dinv_dram")

            # ---- persistent SBUF ----
            iota_sb = per.tile([128, TIL], f32)
            nc.sync.dma_start(iota_sb[:], iota_c[:])
            id_sb = per.tile([128, 128], f32)
            nc.sync.dma_start(id_sb[:], id_c[:])
            onec_sb = per.tile([128, 1], f32)
            nc.sync.dma_start(onec_sb[:], ones_col_c[:])
            oner_sb = per.tile([1, 512], f32)
            nc.sync.dma_start(oner_sb[:], ones_row_c[:])
            dstrel_sb = per.tile([128, TOTC], f32)
            nc.sync.dma_start(dstrel_sb[:], dstrel_t[:])
            batchv_sb = per.tile([128, NPC_PAD // 128], f32)
            nc.sync.dma_start(batchv_sb[:], batchv_t[:])
            Ws_sb = per.tile([2 * D, L, D], f32)
            nc.sync.dma_start(Ws_sb[0:D], Ws_t[:].rearrange("l k m -> k l m"))
            nc.sync.dma_start(Ws_sb[D:2 * D], Ws_t[:].rearrange("l k m -> k l m"))
            bs_sb = per.tile([1, L, D], f32)
            nc.sync.dma_start(bs_sb[:], bs_t[:].rearrange("l m -> () l m"))

            y_nm = per.tile([128, NPC_PAD // 128, D], f32)  # node-major staging
            nc.sync.dma_start(y_nm[:], x_own[:].rearrange("(g p) f -> p g f", p=128))
            x3_aug = per.tile([128, NPC_PAD // 128, D + 1], f32)
            nc.vector.memset(x3_aug[:, :, D:D + 1], 1.0)
            xT = per.tile([D, NPC_PAD], f32)          # current x, feature-major
            dinv_row = per.tile([1, NPC_PAD], f32)
            dinv_bc = per.tile([128, NPC_PAD], f32)   # dinv broadcast across partitions
            zero_sb = per.tile([128, D], f32)
            nc.vector.memset(zero_sb[:], 0.0)
            dinv_nm = per.tile([128, NPC_PAD // 128], f32)

            def build_ind(g, h):
                nbc = int(batch_chunks[g, h])
                cb = int(cbase[g, h])
                ind = wrk.tile([128, MAXCH, TIL], f32, tag="ind")
                if SKIP_IND:
                    nc.vector.memset(ind[:, 0:nbc, :], 0.0)
                    return ind
                nc.vector.tensor_tensor(
                    out=ind[:, 0:nbc, :],
                    in0=iota_sb[:, None, :].to_broadcast([128, nbc, TIL]),
                    in1=dstrel_sb[:, cb:cb + nbc, None].to_broadcast([128, nbc, TIL]),
                    op=ALU.is_equal)
                return ind

            def flags(g, tt, h, j):
                t = g * TPG + tt
                first = h == 0 and j == 0
                last = ((h == 1 and j == plan[t, 1] - 1)
                        or (h == 0 and plan[t, 1] == 0 and j == plan[t, 0] - 1))
                return bool(first), bool(last)

            # ================= degree pass =================
            if DEG_ON_HOST and _runs("deg"):
                nc.sync.dma_start(dinv_nm[:], deg_t[:])
                nc.vector.tensor_scalar_max(out=dinv_nm[:], in0=dinv_nm[:], scalar1=1.0)
                dsq_nm = sml.tile([128, NPC_PAD // 128], f32, tag="dr")
                nc.scalar.activation(out=dsq_nm[:], in_=dinv_nm[:], func=AF.Sqrt)
                nc.vector.reciprocal(out=dinv_nm[:], in_=dsq_nm[:])
            for g in range(NGRP if (_runs("deg") and not SKIP_DEG and not DEG_ON_HOST) else 0):
                deg_ps = ps.tile([1, 512], f32, space="PSUM", tag="tp")
                inds = [build_ind(g, 0), build_ind(g, 1)]
                for tt in range(TPG):
                    t = g * TPG + tt
                    for h in range(2):
                        for j in range(int(plan[t, h])):
                            first, last = flags(g, tt, h, j)
                            jj = int(toff[g, h, tt]) + j
                            nc.tensor.matmul(
                                out=deg_ps[0:1, tt * TIL:(tt + 1) * TIL],
                                lhsT=onec_sb[:, 0:1], rhs=inds[h][:, jj, :],
                                start=first, stop=last)
                dmax = sml.tile([1, 512], f32, tag="dr")
                nc.vector.tensor_scalar_max(out=dmax[:], in0=deg_ps[:], scalar1=1.0)
                dsq = sml.tile([1, 512], f32, tag="dr2")
                nc.scalar.activation(out=dsq[:], in_=dmax[:], func=AF.Sqrt)
                nc.vector.reciprocal(out=dinv_row[:, g * 512:(g + 1) * 512], in_=dsq[:])
            if SKIP_DEG and _runs("deg"):
                nc.vector.memset(dinv_row[:], 1.0)

            if _runs("y0"):
                if DEG_ON_HOST:
                    nc.sync.dma_start(dinv_dram[:].rearrange("g p -> p g"), dinv_nm[:])
                    nc.sync.dma_start(dinv_row[:], dinv_dram[:].rearrange("g p -> () (g p)"))
                else:
                    nc.sync.dma_start(dinv_dram[:].rearrange("g p -> () (g p)"), dinv_row[:])
                    nc.sync.dma_start(dinv_nm[:], dinv_dram[:].rearrange("g p -> p g"))
                # dinv broadcast tiles (feature-major, all 128 partitions)
                for g in range(NGRP):
                    bc_ps = ps.tile([128, 512], f32, space="PSUM", tag="tp")
                    nc.tensor.matmul(out=bc_ps[:], lhsT=oner_sb[0:1, 0:128],
                                     rhs=dinv_row[:, g * 512:(g + 1) * 512],
                                     start=True, stop=True)
                    nc.scalar.copy(out=dinv_bc[:, g * 512:(g + 1) * 512], in_=bc_ps[:])
                # y0 = x * dinv (node-major, in place), export + AllGather
                nc.vector.tensor_tensor(
                    out=y_nm[:], in0=y_nm[:],
                    in1=dinv_nm[:, :, None].to_broadcast([128, NPC_PAD // 128, D]),
                    op=ALU.mult)
                nc.sync.dma_start(y_shard[0][:].rearrange("(g p) f -> p g f", p=128), y_nm[:])
                nc.gpsimd.collective_compute(
                    "AllGather", ALU.bypass, replica_groups=[list(range(C))],
                    ins=[y_shard[0][:]], outs=[y_full[0][:]])

            # ================= layers =================
            _nl = NLAYERS
            if STOP_AFTER in ("setup", "deg", "y0"):
                _nl = 0
            elif STOP_AFTER == "L0":
                _nl = 1
            elif STOP_AFTER == "L1":
                _nl = 2
            pend_inds = None
            for l in range(_nl):
                for g in range(NGRP):
                    agg_ps = ps.tile([128, 512], f32, space="PSUM", tag="agg")
                    msgs = []
                    for h in range(2):
                        nbc = int(batch_chunks[g, h])
                        cb = int(cbase[g, h])
                        nb = nbc * 128
                        gi = wrk.tile([128, MAXCH * 8], mybir.dt.int16, tag="gi")
                        nc.sync.dma_start(gi[:, 0:nb // 16],
                                          gidx_t[:, cb * 8:cb * 8 + nb // 16])
                        m = wrk.tile([128, MAXCH, D], f32, tag="msgs")
                        src_ap = y_full[l][HALF:, :] if h else y_full[l][0:HALF, :]
                        if SKIP_GATHER:
                            nc.vector.memset(m[:, 0:nbc, :], 0.125)
                        else:
                            splits = np.linspace(0, nbc, GATHER_SPLIT + 1).astype(int)
                            for s0, s1 in zip(splits[:-1], splits[1:]):
                                if s1 > s0:
                                    nsub = int(s1 - s0) * 128
                                    nc.gpsimd.dma_gather(
                                        m[:, s0:s1, :], src_ap,
                                        gi[:, s0 * 8:s0 * 8 + nsub // 16],
                                        nsub, nsub, D, single_packet=False)
                        msgs.append(m)
                    if g == 0 and pend_inds is not None:
                        inds = pend_inds
                        pend_inds = None
                    else:
                        inds = [build_ind(g, 0), build_ind(g, 1)]
                    if SKIP_AGG:
                        nc.tensor.matmul(out=agg_ps[0:D, :], lhsT=msgs[0][:, 0, :],
                                         rhs=inds[0][:, 0:8, :].rearrange("p c d -> p (c d)"),
                                         start=True, stop=True)
                        nc.tensor.matmul(out=agg_ps[D:128, :], lhsT=zero_sb[:],
                                         rhs=inds[0][:, 0:8, :].rearrange("p c d -> p (c d)"),
                                         start=True, stop=True, tile_position=(0, D))
                    elif COL_PACK:
                        for tt in range(TPG):
                            t = g * TPG + tt
                            sl_t = slice(tt * TIL, (tt + 1) * TIL)
                            clist = [(h, j) for h in (0, 1)
                                     for j in range(int(plan[t, h]))]
                            npar = [(len(clist) + 1) // 2, len(clist) // 2]
                            cnt_p = [0, 0]
                            for ic, (h, j) in enumerate(clist):
                                p = ic % 2
                                jj = int(toff[g, h, tt]) + j
                                nc.tensor.matmul(
                                    out=agg_ps[D * p:D * p + D, sl_t],
                                    lhsT=msgs[h][:, jj, :], rhs=inds[h][:, jj, :],
                                    start=(cnt_p[p] == 0), stop=(cnt_p[p] == npar[p] - 1),
                                    tile_position=(0, D) if p else None,
                                    skip_group_check=True)
                                cnt_p[p] += 1
                            if npar[1] == 0:
                                nc.tensor.matmul(
                                    out=agg_ps[D:2 * D, sl_t], lhsT=zero_sb[:],
                                    rhs=inds[0][:, int(toff[g, 0, tt]), :],
                                    start=True, stop=True, tile_position=(0, D),
                                    skip_group_check=True)
                    else:
                        for tt in range(TPG):
                            t = g * TPG + tt
                            for h in range(2):
                                for j in range(int(plan[t, h])):
                                    first, last = flags(g, tt, h, j)
                                    jj = int(toff[g, h, tt]) + j
                                    nc.tensor.matmul(
                                        out=agg_ps[0:D, tt * TIL:(tt + 1) * TIL],
                                        lhsT=msgs[h][:, jj, :], rhs=inds[h][:, jj, :],
                                        start=first, stop=last)
                    # epilogue for this 512-node group
                    sl = slice(g * 512, (g + 1) * 512)
                    KT = 2 * D if COL_PACK else D
                    rhs_sb = sml.tile([128, 512], f32, tag="rhs")
                    nc.vector.tensor_tensor(out=rhs_sb[0:D, :], in0=agg_ps[0:D, :],
                                            in1=dinv_bc[0:D, sl], op=ALU.mult)
                    if COL_PACK:
                        nc.vector.tensor_tensor(out=rhs_sb[D:2 * D, :],
                                                in0=agg_ps[D:2 * D, :],
                                                in1=dinv_bc[D:2 * D, sl], op=ALU.mult)
                    tr_ps = ps.tile([D, 512], f32, space="PSUM", tag="tr")
                    if l > 0:
                        nc.tensor.matmul(out=tr_ps[:], lhsT=id_sb[0:D, 0:D],
                                         rhs=xT[:, sl], start=True, stop=False)
                    nc.tensor.matmul(out=tr_ps[:], lhsT=Ws_sb[0:KT, l, :],
                                     rhs=rhs_sb[0:KT, :],
                                     start=(l == 0), stop=False)
                    nc.tensor.matmul(out=tr_ps[:], lhsT=bs_sb[:, l, :], rhs=oner_sb[:],
                                     start=False, stop=True)
                    if LRELU_DECOMP:
                        r_sb = sml.tile([D, 512], f32, tag="lr1", bufs=1)
                        nc.scalar.activation(out=r_sb[:], in_=tr_ps[:], func=AF.Relu)
                        t_sb = sml.tile([D, 512], f32, tag="lr2", bufs=1)
                        nc.scalar.activation(out=t_sb[:], in_=tr_ps[:],
                                             func=AF.Copy, scale=0.01)
                        nc.vector.scalar_tensor_tensor(
                            out=xT[:, sl], in0=r_sb[:], scalar=0.99, in1=t_sb[:],
                            op0=ALU.mult, op1=ALU.add)
                    else:
                        nc.scalar.activation(out=xT[:, sl], in_=tr_ps[:],
                                             func=AF.Lrelu, alpha=0.01)
                    tp_ps = ps.tile([128, 256], f32, space="PSUM", tag="tp")
                    if l < L - 1:
                        yT = sml.tile([D, 512], f32, tag="yT")
                        nc.vector.tensor_tensor(out=yT[:], in0=xT[:, sl],
                                                in1=dinv_bc[0:D, sl], op=ALU.mult)
                        for k in range(4):
                            nc.tensor.transpose(out=tp_ps[:, k * D:(k + 1) * D],
                                                in_=yT[:, k * 128:(k + 1) * 128],
                                                identity=id_sb[0:D, 0:D])
                        nc.scalar.copy(
                            out=y_nm[:, g * 4:(g + 1) * 4, :],
                            in_=tp_ps[:].rearrange("p (g f) -> p g f", f=D))
                    else:
                        for k in range(4):
                            nc.tensor.transpose(out=tp_ps[:, k * D:(k + 1) * D],
                                                in_=xT[:, g * 512 + k * 128: g * 512 + (k + 1) * 128],
                                                identity=id_sb[0:D, 0:D])
                        nc.scalar.copy(
                            out=x3_aug[:, g * 4:(g + 1) * 4, 0:D],
                            in_=tp_ps[:].rearrange("p (g f) -> p g f", f=D))
                if l < L - 1:
                    nc.sync.dma_start(
                        y_shard[l + 1][:].rearrange("(g p) f -> p g f", p=128), y_nm[:])
                    pend_inds = [build_ind(0, 0), build_ind(0, 1)]
                    nc.gpsimd.collective_compute(
                        "AllGather", ALU.bypass, replica_groups=[list(range(C))],
                        ins=[y_shard[l + 1][:]], outs=[y_full[l + 1][:]])

            # ================= pooling =================
            if _runs("pool"):
                NCG = NPC_PAD // 128  # 52
                pind = wrk.tile([128, NCG, G], f32, tag="ind")
                nc.vector.tensor_tensor(
                    out=pind[:],
                    in0=iota_sb[:, None, :].to_broadcast([128, NCG, G]),
                    in1=batchv_sb[:, :, None].to_broadcast([128, NCG, G]),
                    op=ALU.is_equal)
                pool_ps = ps.tile([D + 1, G], f32, space="PSUM", tag="tr")
                for t in range(NCG):
                    nc.tensor.matmul(out=pool_ps[:], lhsT=x3_aug[:, t, :], rhs=pind[:, t, :],
                                     start=(t == 0), stop=(t == NCG - 1))
                pool_sb = sml.tile([D + 1, G], f32, tag="dr")
                nc.vector.tensor_copy(out=pool_sb[:], in_=pool_ps[:])
                nc.sync.dma_start(out_t[:], pool_sb[:])

    nc.compile()
    return nc


def kernel(x, edge_index, batch, Ws, bs):
    from concourse.bass_utils import run_bass_kernel_spmd

    x = np.asarray(x, np.float32)
    Ws_np = np.asarray(Ws, np.float32)
    bs_np = np.asarray(bs, np.float32)

    xs, bvs, gidx_w, dstrel_w, batch_chunks, plan, tot_chunks, degs = _host_prep(
        x, edge_index, batch)

    key = (batch_chunks.tobytes(), plan.tobytes())
    if key not in _BUILD_CACHE:
        _BUILD_CACHE[key] = _build(batch_chunks, plan, tot_chunks)
    nc = _BUILD_CACHE[key]

    in_maps = []
    for c in range(C):
        in_maps.append({
            "x_own": xs[c],
            "gidx": np.ascontiguousarray(gidx_w[c]),
            "dstrel": np.ascontiguousarray(dstrel_w[c]),
            "batchv": np.ascontiguousarray(bvs[c]),
            "Ws": Ws_np,
            "bs": bs_np,
            "deg_own": degs[c],
        })
    res = None
    for attempt in range(3):
        try:
            res = run_bass_kernel_spmd(nc, in_maps, core_ids=list(range(C)),
                                       trace=TRACE)
            break
        except Exception:
            if attempt == 2:
                raise
            import time
            time.sleep(5.0)
    global LAST_RESULT
    LAST_RESULT = res

    total = np.zeros((D + 1, G), np.float64)
    for c in range(C):
        total += res.results[c]["out_partial"].astype(np.float64)
    sums = total[:D]                    # [feat, graph]
    counts = np.maximum(total[D], 1.0)  # [graph]
    pooled = (sums / counts[None, :]).T.astype(np.float32)
    return pooled

